# revision 10
# baseline (speedup 1.0000x reference)
"""Trainium2 Bass kernel for nn_AutoregressivePPRM.

Model (per sequence row): 24-step autoregressive GRU (input_size=1, hidden=256)
whose scalar input at each step is the previous step's prediction
pred = (h @ Wl.T + bl) @ Wh.T + bh  -- affine in h. We exploit that to fold the
input path into the recurrent weights:

  x_{t+1} = w_e @ h_t + b_e            (w_e = Wh@Wl [1,256], b_e scalar)
  gi_rz   = W_ih_rz * x + b_ih_rz  =>  folded: W_rz_eff = W_hh_rz + W_ih_rz @ w_e
  inn     = w_in * x_t + b_in          computed from a PE-replicated x row:
            px~ = (w_e (x) 1_128) @ h  gives x on all 128 partitions, so
            inn is a cheap per-partition tensor_scalar, not a matmul.

Layout: hidden dim on partitions, B*N rows on the free axis, bf16 storage for
h / gates / weights (fp32 PSUM accumulation), which unlocks the DVE 2x mode
for the elementwise GRU state update. z-gate sigmoid is pair-fused across two
PSUM banks with its bias pre-written into PSUM by a Pool copy; r-gate sigmoids
run in-place on PSUM with per-partition bias.
Data parallel: 16000 rows sharded 2000/core across 8 cores; weights replicated.
"""

import os

import numpy as np

import concourse.bass as bass
import concourse.tile as tile
from concourse import bacc
from concourse import mybir
from concourse.bass_utils import run_bass_kernel_spmd

B, N, D, HOR, BOT = 32, 500, 256, 24, 8
NCORES = 8
ROWS = B * N // NCORES          # 2000 rows per core
RT = 4                          # row tiles per core
W = ROWS // RT                  # 500 columns per row tile
KT = D // 128                   # 2 contraction tiles

F32 = mybir.dt.float32
F32R = mybir.dt.float32r
BF16 = mybir.dt.bfloat16

# bf16 matmul weights, packed into one [128, PK_TOT] tensor
PK_OFF = {}
_c = 0
for _n, _w in [("w_rz", KT * 512), ("w_rz1", KT * 512), ("w_hn", KT * 256),
               ("w_er", KT * 128), ("wg1", KT * BOT), ("wirz", 512),
               ("wg2", HOR)]:
    PK_OFF[_n] = _c
    _c += _w
PK_TOT = _c

# f32 per-partition scalars (biases), packed into [128, BP_TOT]
BP_OFF = {}
_c = 0
for _n, _w in [("br", 2), ("br1", 2), ("bz", 2), ("bz1", 2), ("bhn", KT),
               ("win", KT), ("binn", KT), ("bg1", 1), ("bg2", 1)]:
    BP_OFF[_n] = _c
    _c += _w
BP_TOT = _c


def build_nc():
    nc = bacc.Bacc()

    dram = {}

    def param(name, shape, out=False, dt=None):
        dram[name] = nc.declare_dram_parameter(
            name, list(shape), dt or F32, isOutput=out
        )
        return dram[name]

    hT_d = param("hT", (KT, 128, ROWS), dt=BF16)
    wpk_d = param("wpk", (128, PK_TOT), dt=BF16)
    bpk_d = param("bpk", (128, BP_TOT))
    lvrep_d = param("lvrep", (128, ROWS), dt=BF16)
    dlv_d = param("dlv", (1, HOR + ROWS), dt=F32R)
    out_d = param("out", (HOR, ROWS), out=True)

    with tile.TileContext(nc, trace_sim=bool(os.environ.get('KTRACE')),
                          pool_alloc_mode='stack') as tc:
        with (
            tc.tile_pool(name="wp", bufs=1) as wp,
            tc.tile_pool(name="hp", bufs=1) as hp,
            tc.tile_pool(name="gp", bufs=3) as gp,
            tc.tile_pool(name="pp", bufs=1, space="PSUM") as pp,
        ):
            # ---- load weights/biases ----
            wpk = wp.tile([128, PK_TOT], BF16, tag="wpk")
            nc.sync.dma_start(out=wpk, in_=wpk_d[:])
            O = PK_OFF
            w_rz = wpk[:, O["w_rz"]:O["w_rz"] + KT * 512].rearrange(
                "p (k n) -> p k n", k=KT)
            w_rz1 = wpk[:, O["w_rz1"]:O["w_rz1"] + KT * 512].rearrange(
                "p (k n) -> p k n", k=KT)
            w_hn = wpk[:, O["w_hn"]:O["w_hn"] + KT * 256].rearrange(
                "p (k n) -> p k n", k=KT)
            w_er = wpk[:, O["w_er"]:O["w_er"] + KT * 128].rearrange(
                "p (k n) -> p k n", k=KT)
            wg1 = wpk[:, O["wg1"]:O["wg1"] + KT * BOT].rearrange(
                "p (k n) -> p k n", k=KT)
            wirz = wpk[0:1, O["wirz"]:O["wirz"] + 512]
            wg2 = wpk[0:BOT, O["wg2"]:O["wg2"] + HOR]

            bpk = wp.tile([128, BP_TOT], F32, tag="bpk")
            nc.sync.dma_start(out=bpk, in_=bpk_d[:])
            OB = BP_OFF
            br = bpk[:, OB["br"]:OB["br"] + 2]
            br1 = bpk[:, OB["br1"]:OB["br1"] + 2]
            bz = bpk[:, OB["bz"]:OB["bz"] + 2]
            bz1 = bpk[:, OB["bz1"]:OB["bz1"] + 2]
            bhn = bpk[:, OB["bhn"]:OB["bhn"] + KT]
            win = bpk[:, OB["win"]:OB["win"] + KT]
            binn = bpk[:, OB["binn"]:OB["binn"] + KT]
            bg1 = bpk[0:BOT, OB["bg1"]:OB["bg1"] + 1]
            bg2 = bpk[0:HOR, OB["bg2"]:OB["bg2"] + 1]

            lvrep = wp.tile([128, ROWS], BF16, tag="lvrep")
            nc.scalar.dma_start(out=lvrep, in_=lvrep_d[:])
            dlv = wp.tile([1, HOR + ROWS], F32R, tag="dlv")
            nc.sync.dma_start(out=dlv, in_=dlv_d[:])
            wdk = dlv[0:1, 0:HOR]
            lvr = dlv[0:1, HOR:HOR + ROWS]

            # ---- hidden state (= features, transposed), per row tile ----
            h = []
            for r in range(RT):
                h_r = hp.tile([128, KT, W], BF16, tag=f"h{r}")
                for k in range(KT):
                    nc.gpsimd.dma_start(
                        out=h_r[:, k, :], in_=hT_d[k, :, r * W:(r + 1) * W]
                    )
                h.append(h_r)

            preds = [hp.tile([HOR, W], BF16, tag=f"preds{r}", name=f"preds{r}")
                     for r in range(RT)]
            gate = hp.tile([HOR, ROWS], F32, tag="gate")

            SIG = mybir.ActivationFunctionType.Sigmoid
            TANH = mybir.ActivationFunctionType.Tanh
            ADD = mybir.AluOpType.add
            SUB = mybir.AluOpType.subtract
            MULT = mybir.AluOpType.mult
            MAXX = mybir.AluOpType.max

            # ---- prologue: mixing gate from h0 (also warms up the PE) ----
            g1 = hp.tile([BOT, ROWS], BF16, tag="g1")
            for r in range(RT):
                cols = slice(r * W, (r + 1) * W)
                pg1 = pp.tile([BOT, W], F32, tag="hn0")
                for k in range(KT):
                    nc.tensor.matmul(
                        pg1[:], (wg1[:, k, :]), (h[r][:, k, :]),
                        start=(k == 0), stop=(k == KT - 1),
                    )
                # relu(pg1 + bg1) on DVE (gpsimd cannot access PSUM)
                nc.vector.tensor_scalar(out=g1[:, cols], in0=pg1[:],
                                        scalar1=bg1[:, 0:1], scalar2=0.0,
                                        op0=ADD, op1=MAXX)
                pg2 = pp.tile([HOR, W], F32, tag="px")
                nc.tensor.matmul(pg2[:], (wg2[:]), (g1[:, cols]),
                                 start=True, stop=True)
                nc.scalar.activation(gate[:, cols], pg2[:], SIG, bias=bg2[:, 0:1])

            # ---- 24 GRU steps ----
            for u in range(1, HOR + 1):
                first = u == 1
                w_cur = w_rz1 if first else w_rz
                br_cur = br1 if first else br
                bz_cur = bz1 if first else bz
                zsel = 0 if first else 1
                for r in range(RT):
                    cols = slice(r * W, (r + 1) * W)
                    # r/z gates: one 1-bank psum tile per 128-row block so
                    # each frees as soon as its consumer has read it
                    prz = [pp.tile([128, 512], F32, tag=f"rz{m}", name=f"prz{m}")
                           for m in range(4)]
                    for m in range(4):
                        dst = prz[m][:, 0:W]
                        for k in range(KT):
                            nc.tensor.matmul(
                                dst,
                                (w_cur[:, k, m * 128:(m + 1) * 128]),
                                (h[r][:, k, :]),
                                start=(k == 0),
                                stop=(k == KT - 1 and not first),
                            )
                        if first:
                            nc.tensor.matmul(
                                dst,
                                (wirz[0:1, m * 128:(m + 1) * 128]),
                                (lvrep[0:1, cols]), start=False, stop=True,
                            )
                    # r-gate: per-bank sigmoid -> bf16 SBUF (walrus forbids
                    # two PSUM inputs on one op, so r can't stay in PSUM)
                    r_sb = gp.tile([128, KT, W], BF16, tag="rs")
                    for m in range(2):
                        nc.scalar.activation(r_sb[:, m, :], prz[m][:, 0:W],
                                             SIG, bias=br_cur[:, m:m + 1])
                    # z-gate: per-bank sigmoid -> bf16 SBUF
                    z_sb = gp.tile([128, KT, W], BF16, tag="zs")
                    for m in range(2):
                        nc.scalar.activation(z_sb[:, m, :], prz[2 + m][:, 0:W],
                                             SIG, bias=bz_cur[:, m:m + 1])

                    # hn matmuls + replicated-x matmul
                    phn = [pp.tile([128, 512], F32, tag=f"hn{m}", name=f"phn{m}")
                           for m in range(KT)]
                    for m in range(KT):
                        for k in range(KT):
                            nc.tensor.matmul(
                                phn[m][:, 0:W],
                                (w_hn[:, k, m * 128:(m + 1) * 128]),
                                (h[r][:, k, :]),
                                start=(k == 0), stop=(k == KT - 1),
                            )
                    if not first:
                        px = pp.tile([128, 512], F32, tag="px")
                        for k in range(KT):
                            nc.tensor.matmul(
                                px[:, 0:W], (w_er[:, k, :]), (h[r][:, k, :]),
                                start=(k == 0), stop=(k == KT - 1),
                            )
                        # x~ = px + b_e -> bf16 SBUF (x replicated on 128 parts)
                        xt = gp.tile([128, W], BF16, tag="xt")
                        nc.vector.tensor_scalar(
                            out=xt, in0=px[:, 0:W],
                            scalar1=BE_CONST[0], scalar2=0.0, op0=ADD, op1=ADD)
                        # pred_{u-1} = x_u  (row 0 of the replicated x~);
                        # engines can't write at a partition offset -> DMA
                        nc.sync.dma_start(out=preds[r][u - 2:u - 1, :],
                                          in_=xt[0:1, :])
                    else:
                        xt = lvrep[:, cols]

                    # t_k = (phn_k + bhn_k) * r_k   (r lives in PSUM)
                    t_sb = gp.tile([128, KT, W], BF16, tag="t")
                    for k in range(KT):
                        nc.vector.scalar_tensor_tensor(
                            out=t_sb[:, k, :], in0=phn[k][:, 0:W],
                            scalar=bhn[:, k:k + 1], in1=r_sb[:, k, :],
                            op0=ADD, op1=MULT,
                        )
                    # q_k = w_in_k * x~ + b_in_k   (all-SBUF bf16: DVE 4x mode)
                    q_sb = gp.tile([128, KT, W], BF16, tag="q")
                    for k in range(KT):
                        nc.vector.tensor_scalar(
                            out=q_sb[:, k, :], in0=xt,
                            scalar1=win[:, k:k + 1], scalar2=binn[:, k:k + 1],
                            op0=MULT, op1=ADD)
                    t2 = gp.tile([128, KT, W], BF16, tag="t2")
                    nc.vector.tensor_tensor(out=t2, in0=t_sb, in1=q_sb, op=ADD)
                    ncand = gp.tile([128, KT, W], BF16, tag="nc")
                    nc.scalar.activation(ncand[:, :, :], t2[:, :, :],
                                         TANH, bias=0.0)
                    # h = ncand + z * (h - ncand)   (SBUF-only: gpsimd)
                    d_sb = gp.tile([128, KT, W], BF16, tag="d")
                    nc.gpsimd.tensor_sub(d_sb[:], h[r][:, :, :], ncand[:])
                    nc.gpsimd.tensor_mul(d_sb[:], d_sb[:], z_sb[:])
                    nc.gpsimd.tensor_add(h[r][:, :, :], ncand[:], d_sb[:])

            # ---- final pred_24 ----
            for r in range(RT):
                px = pp.tile([128, 512], F32, tag="px")
                for k in range(KT):
                    nc.tensor.matmul(
                        px[:, 0:W], (w_er[:, k, :]), (h[r][:, k, :]),
                        start=(k == 0), stop=(k == KT - 1),
                    )
                xtf = gp.tile([1, W], BF16, tag="xtf")
                nc.vector.tensor_scalar(
                    out=xtf, in0=px[0:1, 0:W],
                    scalar1=BE_CONST[0], scalar2=0.0, op0=ADD, op1=ADD)
                nc.sync.dma_start(out=preds[r][HOR - 1:HOR, :], in_=xtf)


            # ---- epilogue: out = decay + gate * (preds - decay) ----
            for r in range(RT):
                cols = slice(r * W, (r + 1) * W)
                pdec = pp.tile([HOR, W], F32, tag="hn0")
                nc.tensor.matmul(pdec[:], (wdk[:]), (lvr[0:1, cols]),
                                 start=True, stop=True)
                td = gp.tile([HOR, W], F32, tag="td")
                nc.vector.tensor_tensor(out=td, in0=preds[r][:], in1=pdec[:],
                                        op=SUB)
                nc.gpsimd.tensor_mul(td[:], td[:], gate[:, cols])
                out_sb = gp.tile([HOR, W], F32, tag="osb")
                nc.vector.tensor_tensor(out=out_sb, in0=td, in1=pdec[:], op=ADD)
                nc.sync.dma_start(out=out_d[:, cols], in_=out_sb)

    nc.finalize()
    return nc


# b_e is a host-computed constant baked into the program as an immediate; the
# module-level cell lets build_nc read it before kernel() computes it.
BE_CONST = [0.0]

_NC_CACHE = None


def _get_nc():
    global _NC_CACHE
    if _NC_CACHE is None:
        _NC_CACHE = build_nc()
    return _NC_CACHE


def kernel(features, last_value, W_ih, W_hh, b_ih, b_hh, Wl, bl, Wh, bh,
           Wg1, bg1, Wg2, bg2, log_decay):
    import ml_dtypes
    NBF = ml_dtypes.bfloat16
    features = np.asarray(features, np.float32)
    last_value = np.asarray(last_value, np.float32)
    f64 = lambda a: np.asarray(a, np.float64)
    W_ih, W_hh, b_ih, b_hh = map(f64, (W_ih, W_hh, b_ih, b_hh))
    Wl, bl, Wh, bh = map(f64, (Wl, bl, Wh, bh))
    Wg1, bg1, Wg2, bg2 = map(f64, (Wg1, bg1, Wg2, bg2))

    w_e = (Wh @ Wl)[0]                      # [256]
    b_e = float((Wh @ bl + bh)[0])
    BE_CONST[0] = b_e
    W_rz_eff = W_hh[0:512] + W_ih[0:512] @ w_e[None, :]
    b_rz_eff = b_hh[0:512] + b_ih[0:512] + W_ih[0:512, 0] * b_e
    b_rz1 = b_hh[0:512] + b_ih[0:512]
    w_in = W_ih[512:768, 0]
    b_in = b_ih[512:768]
    t = np.arange(1, HOR + 1, dtype=np.float64)
    decay_curve = np.exp(-np.exp(float(log_decay)) * t)

    def pack_kpn(arr_t):  # [D, M] -> [128, KT*M] laid out (p, k*M+n)
        kt = arr_t.reshape(KT, 128, -1)
        return np.transpose(kt, (1, 0, 2)).reshape(128, -1)

    pk = np.zeros((128, PK_TOT), NBF)
    bp = np.zeros((128, BP_TOT), np.float32)
    O = PK_OFF
    OB = BP_OFF

    def put(name, block):
        block = np.asarray(block, NBF)
        pk[:block.shape[0], O[name]:O[name] + block.shape[1]] = block

    def putb(name, block):
        block = np.asarray(block, np.float32)
        bp[:block.shape[0], OB[name]:OB[name] + block.shape[1]] = block

    put("w_rz", pack_kpn(W_rz_eff.T))
    put("w_rz1", pack_kpn(W_hh[0:512].T))
    put("w_hn", pack_kpn(W_hh[512:768].T))
    put("w_er", pack_kpn(np.repeat(w_e[:, None], 128, axis=1)))
    put("wg1", pack_kpn(Wg1.T))
    put("wirz", W_ih[0:512, 0][None, :])
    put("wg2", Wg2.T)

    putb("br", b_rz_eff[0:256].reshape(2, 128).T)
    putb("br1", b_rz1[0:256].reshape(2, 128).T)
    putb("bz", b_rz_eff[256:512].reshape(2, 128).T)
    putb("bz1", b_rz1[256:512].reshape(2, 128).T)
    putb("bhn", b_hh[512:768].reshape(KT, 128).T)
    putb("win", w_in.reshape(KT, 128).T)
    putb("binn", b_in.reshape(KT, 128).T)
    putb("bg1", bg1[:, None])
    putb("bg2", bg2[:, None])

    dlv_row = np.zeros((1, HOR + ROWS), np.float32)
    dlv_row[0, 0:HOR] = decay_curve

    feat_flat = features.reshape(B * N, D)
    lv_flat = last_value.reshape(B * N)
    in_maps = []
    for i in range(NCORES):
        rows = slice(i * ROWS, (i + 1) * ROWS)
        dlv_i = dlv_row.copy()
        dlv_i[0, HOR:] = lv_flat[rows]
        m = {
            "hT": np.ascontiguousarray(
                feat_flat[rows].T.reshape(KT, 128, ROWS)).astype(NBF),
            "wpk": pk,
            "bpk": bp,
            "lvrep": np.repeat(lv_flat[rows][None, :], 128, axis=0).astype(NBF),
            "dlv": dlv_i,
        }
        in_maps.append(m)

    nc = _get_nc()
    try:
        res = run_bass_kernel_spmd(nc, in_maps, core_ids=list(range(NCORES)))
    except Exception:
        res = run_bass_kernel_spmd(nc, in_maps, core_ids=list(range(NCORES)))
    global LAST_RESULT
    LAST_RESULT = res
    out = np.concatenate([r["out"].T for r in res.results], axis=0)
    return np.ascontiguousarray(out.reshape(B, N, HOR), np.float32)


LAST_RESULT = None


# revision 29
# speedup vs baseline: 1.0627x; 1.0627x over previous
"""Trainium2 Bass kernel for nn_AutoregressivePPRM.

Model (per sequence row): 24-step autoregressive GRU (input_size=1, hidden=256)
whose scalar input at each step is the previous step's prediction
pred = (h @ Wl.T + bl) @ Wh.T + bh  -- affine in h. We exploit that to fold the
input path into the recurrent weights:

  x_{t+1} = w_e @ h_t + b_e            (w_e = Wh@Wl [1,256], b_e scalar)
  gi_rz   = W_ih_rz * x + b_ih_rz  =>  folded: W_rz_eff = W_hh_rz + W_ih_rz @ w_e
  inn     = w_in * x_t + b_in          computed from a PE-replicated x row:
            px~ = (w_e (x) 1_128) @ h  gives x on all 128 partitions, so
            inn is a cheap per-partition tensor_scalar, not a matmul.

Layout: hidden dim on partitions, B*N rows on the free axis, bf16 storage for
h / gates / weights (fp32 PSUM accumulation), which unlocks the DVE 2x mode
for the elementwise GRU state update. z-gate sigmoid is pair-fused across two
PSUM banks with its bias pre-written into PSUM by a Pool copy; r-gate sigmoids
run in-place on PSUM with per-partition bias.
Data parallel: 16000 rows sharded 2000/core across 8 cores; weights replicated.
"""

import os

import numpy as np

import concourse.bass as bass
import concourse.tile as tile
from concourse import bacc
from concourse import mybir
from concourse.bass_utils import run_bass_kernel_spmd

B, N, D, HOR, BOT = 32, 500, 256, 24, 8
NCORES = 8
ROWS = B * N // NCORES          # 2000 rows per core
RT = 4                          # row tiles per core
W = ROWS // RT                  # 500 columns per row tile
KT = D // 128                   # 2 contraction tiles

F32 = mybir.dt.float32
F32R = mybir.dt.float32r
BF16 = mybir.dt.bfloat16

# bf16 matmul weights, packed into one [128, PK_TOT] tensor
PK_OFF = {}
_c = 0
for _n, _w in [("wg1", KT * BOT), ("w_rz1", KT * 512), ("w_hn", KT * 256),
               ("wirz", 512), ("wib1", 256), ("brb", 256), ("one", W),
               ("w_er", KT * 128), ("wg2", HOR), ("w_rz", KT * 512)]:
    PK_OFF[_n] = _c
    _c += _w
PK_TOT = _c
PK_CUT = PK_OFF["w_er"]   # first DMA chunk: prologue + step-1 weights

# f32 per-partition scalars (biases), packed into [128, BP_TOT]
BP_OFF = {}
_c = 0
for _n, _w in [("br", 2), ("br1", 2), ("bz", 2), ("bz1", 2), ("bhn", KT),
               ("win", KT), ("binn", KT), ("bg1", 1), ("bg2", 1)]:
    BP_OFF[_n] = _c
    _c += _w
BP_TOT = _c


def build_nc():
    nc = bacc.Bacc()

    dram = {}

    def param(name, shape, out=False, dt=None):
        dram[name] = nc.declare_dram_parameter(
            name, list(shape), dt or F32, isOutput=out
        )
        return dram[name]

    hT_d = param("hT", (KT, 128, ROWS), dt=BF16)
    wpk_d = param("wpk", (128, PK_TOT), dt=BF16)
    bpk_d = param("bpk", (128, BP_TOT))
    lvrep_d = param("lvrep", (128, ROWS), dt=BF16)
    lv1_d = param("lv1", (2, ROWS), dt=BF16)
    dlv_d = param("dlv", (1, HOR + ROWS), dt=F32R)
    out_d = param("out", (HOR, ROWS), out=True)

    with tile.TileContext(nc, trace_sim=bool(os.environ.get('KTRACE')),
                          pool_alloc_mode='stack') as tc:
        with (
            tc.tile_pool(name="wp", bufs=1) as wp,
            tc.tile_pool(name="hp", bufs=1) as hp,
            tc.tile_pool(name="gp", bufs=5) as gp,
            tc.tile_pool(name="pp", bufs=1, space="PSUM") as pp,
        ):
            # ---- load weights/biases ----
            wpk = wp.tile([128, PK_TOT], BF16, tag="wpk")
            nc.sync.dma_start(out=wpk[:, 0:PK_CUT], in_=wpk_d[:, 0:PK_CUT])
            nc.scalar.dma_start(out=wpk[:, PK_CUT:], in_=wpk_d[:, PK_CUT:])
            O = PK_OFF
            w_rz = wpk[:, O["w_rz"]:O["w_rz"] + KT * 512].rearrange(
                "p (k n) -> p k n", k=KT)
            w_rz1 = wpk[:, O["w_rz1"]:O["w_rz1"] + KT * 512].rearrange(
                "p (k n) -> p k n", k=KT)
            w_hn = wpk[:, O["w_hn"]:O["w_hn"] + KT * 256].rearrange(
                "p (k n) -> p k n", k=KT)
            w_er = wpk[:, O["w_er"]:O["w_er"] + KT * 128].rearrange(
                "p (k n) -> p k n", k=KT)
            wg1 = wpk[:, O["wg1"]:O["wg1"] + KT * BOT].rearrange(
                "p (k n) -> p k n", k=KT)
            wirz = wpk[0:1, O["wirz"]:O["wirz"] + 512]
            brb = wpk[0:1, O["brb"]:O["brb"] + 256]
            wib1 = wpk[0:2, O["wib1"]:O["wib1"] + 256]
            one_row = wpk[0:1, O["one"]:O["one"] + W]
            wg2 = wpk[0:BOT, O["wg2"]:O["wg2"] + HOR]

            # ---- hidden state (= features, transposed), per row tile ----
            h = [hp.tile([128, KT, W], BF16, tag=f"h{r}", name=f"h{r}")
                 for r in range(RT)]

            def load_h(r):
                for k in range(KT):
                    nc.gpsimd.dma_start(
                        out=h[r][:, k, :], in_=hT_d[k, :, r * W:(r + 1) * W]
                    )

            load_h(0)
            bpk = wp.tile([128, BP_TOT], F32, tag="bpk")
            nc.gpsimd.dma_start(out=bpk, in_=bpk_d[:])
            lvrep = wp.tile([128, ROWS], BF16, tag="lvrep")
            nc.gpsimd.dma_start(out=lvrep, in_=lvrep_d[:])
            lv1 = wp.tile([2, ROWS], BF16, tag="lv1")
            nc.gpsimd.dma_start(out=lv1, in_=lv1_d[:])
            for r in range(1, RT):
                load_h(r)
            dlv = wp.tile([1, HOR + ROWS], F32R, tag="dlv")
            nc.gpsimd.dma_start(out=dlv, in_=dlv_d[:])
            OB = BP_OFF
            br = bpk[:, OB["br"]:OB["br"] + 2]
            br1 = bpk[:, OB["br1"]:OB["br1"] + 2]
            bz = bpk[:, OB["bz"]:OB["bz"] + 2]
            bz1 = bpk[:, OB["bz1"]:OB["bz1"] + 2]
            bhn = bpk[:, OB["bhn"]:OB["bhn"] + KT]
            win = bpk[:, OB["win"]:OB["win"] + KT]
            binn = bpk[:, OB["binn"]:OB["binn"] + KT]
            bg1 = bpk[0:BOT, OB["bg1"]:OB["bg1"] + 1]
            bg2 = bpk[0:HOR, OB["bg2"]:OB["bg2"] + 1]


            wdk = dlv[0:1, 0:HOR]
            lvr = dlv[0:1, HOR:HOR + ROWS]


            preds = [hp.tile([HOR - 1, W], BF16, tag=f"preds{r}",
                             name=f"preds{r}") for r in range(RT)]
            gate = hp.tile([HOR, ROWS], F32, tag="gate")

            SIG = mybir.ActivationFunctionType.Sigmoid
            TANH = mybir.ActivationFunctionType.Tanh
            ADD = mybir.AluOpType.add
            SUB = mybir.AluOpType.subtract
            MULT = mybir.AluOpType.mult
            MAXX = mybir.AluOpType.max

            # ---- prologue: mixing gate from h0 (also warms up the PE) ----
            g1 = hp.tile([BOT, ROWS], BF16, tag="g1")
            bmix = hp.tile([HOR, ROWS], F32, tag="bmix")
            g23 = hp.tile([1, ROWS], F32, tag="g23")
            b23 = hp.tile([1, ROWS], F32, tag="b23")
            ncF = [hp.tile([128, KT, W], BF16, tag=f"ncF{r}", name=f"ncF{r}")
                   for r in range(RT)]
            zdF = [hp.tile([128, KT, W], BF16, tag=f"zdF{r}", name=f"zdF{r}")
                   for r in range(RT)]
            for r in range(RT):
                cols = slice(r * W, (r + 1) * W)
                pg1 = pp.tile([BOT, W], F32, tag="hn0")
                for k in range(KT):
                    nc.tensor.matmul(
                        pg1[:], (wg1[:, k, :]), (h[r][:, k, :]),
                        start=(k == 0), stop=(k == KT - 1),
                    )
                # relu(pg1 + bg1) on DVE (gpsimd cannot access PSUM)
                nc.vector.tensor_scalar(out=g1[:, cols], in0=pg1[:],
                                        scalar1=bg1[:, 0:1], scalar2=0.0,
                                        op0=ADD, op1=MAXX)
                pg2 = pp.tile([HOR, W], F32, tag="px")
                nc.tensor.matmul(pg2[:], (wg2[:]), (g1[:, cols]),
                                 start=True, stop=True)
                nc.scalar.activation(gate[:, cols], pg2[:], SIG, bias=bg2[:, 0:1])
                pdec = pp.tile([HOR, W], F32, tag="pd")
                nc.tensor.matmul(pdec[:], (wdk[:]), (lvr[0:1, cols]),
                                 start=True, stop=True)
                # B = (1 - gate) * decay, finale then needs only preds
                gm = gp.tile([HOR, W], F32, tag="gm")
                nc.vector.tensor_scalar(out=gm, in0=gate[:, cols],
                                        scalar1=-1.0, scalar2=-1.0,
                                        op0=MULT, op1=SUB)
                nc.vector.tensor_tensor(out=bmix[:, cols], in0=gm,
                                        in1=pdec[:], op=MULT)

            # ---- 24 GRU steps ----
            for u in range(1, HOR + 1):
                first = u == 1
                w_cur = w_rz1 if first else w_rz
                br_cur = br1 if first else br
                bz_cur = bz1 if first else bz
                zsel = 0 if first else 1
                for r in range(RT):
                    cols = slice(r * W, (r + 1) * W)
                    # r/z gates: one 1-bank psum tile per 128-row block so
                    # each frees as soon as its consumer has read it
                    prz = [pp.tile([128, 512], F32, tag=f"rz{m}", name=f"prz{m}")
                           for m in range(4)]
                    for m in range(4):
                        dst = prz[m][:, 0:W]
                        for k in range(KT):
                            nc.tensor.matmul(
                                dst,
                                (w_cur[:, k, m * 128:(m + 1) * 128]),
                                (h[r][:, k, :]),
                                start=(k == 0),
                                stop=(k == KT - 1 and not first),
                            )
                        if first:
                            nc.tensor.matmul(
                                dst,
                                (wirz[0:1, m * 128:(m + 1) * 128]),
                                (lvrep[0:1, cols]), start=False, stop=True,
                            )
                    # r-gate: per-bank sigmoid -> bf16 SBUF (walrus forbids
                    # two PSUM inputs on one op, so r can't stay in PSUM)
                    r_sb = gp.tile([128, KT, W], BF16, tag="rs")
                    for m in range(2):
                        nc.scalar.activation(r_sb[:, m, :], prz[m][:, 0:W],
                                             SIG, bias=br_cur[:, m:m + 1])
                    # z-gate: per-bank sigmoid -> bf16 SBUF
                    z_sb = gp.tile([128, KT, W], BF16, tag="zs")
                    for m in range(2):
                        nc.scalar.activation(z_sb[:, m, :], prz[2 + m][:, 0:W],
                                             SIG, bias=bz_cur[:, m:m + 1])

                    # hn matmuls + replicated-x matmul
                    phn = [pp.tile([128, 512], F32, tag=f"hn{m}", name=f"phn{m}")
                           for m in range(KT)]
                    for m in range(KT):
                        for k in range(KT):
                            nc.tensor.matmul(
                                phn[m][:, 0:W],
                                (w_hn[:, k, m * 128:(m + 1) * 128]),
                                (h[r][:, k, :]),
                                start=(k == 0), stop=(k == KT - 1),
                            )
                    if not first:
                        px = pp.tile([128, 512], F32, tag="px")
                        for k in range(KT):
                            nc.tensor.matmul(
                                px[:, 0:W], (w_er[:, k, :]), (h[r][:, k, :]),
                                start=(k == 0), stop=(k == KT - 1),
                            )
                        # x~ = px + b_e -> bf16 SBUF (x replicated on 128 parts)
                        xt = gp.tile([128, W], BF16, tag="xt")
                        nc.vector.tensor_scalar(
                            out=xt, in0=px[:, 0:W],
                            scalar1=BE_CONST[0], scalar2=0.0, op0=ADD, op1=ADD)
                        # pred_{u-1} = x_u  (row 0 of the replicated x~);
                        # engines can't write at a partition offset -> DMA
                        nc.sync.dma_start(out=preds[r][u - 2:u - 1, :],
                                          in_=xt[0:1, :])
                    else:
                        xt = lvrep[:, cols]

                    # t_k = (phn_k + bhn_k) * r_k   (r lives in PSUM)
                    t_sb = gp.tile([128, KT, W], BF16, tag="t")
                    for k in range(KT):
                        nc.vector.scalar_tensor_tensor(
                            out=t_sb[:, k, :], in0=phn[k][:, 0:W],
                            scalar=bhn[:, k:k + 1], in1=r_sb[:, k, :],
                            op0=ADD, op1=MULT,
                        )
                    # q_k = w_in_k * x~ + b_in_k   (all-SBUF bf16: DVE 4x mode)
                    q_sb = gp.tile([128, KT, W], BF16, tag="q")
                    for k in range(KT):
                        nc.vector.tensor_scalar(
                            out=q_sb[:, k, :], in0=xt,
                            scalar1=win[:, k:k + 1], scalar2=binn[:, k:k + 1],
                            op0=MULT, op1=ADD)
                    t2 = gp.tile([128, KT, W], BF16, tag="t2")
                    nc.vector.tensor_tensor(out=t2, in0=t_sb, in1=q_sb, op=ADD)
                    ncand = gp.tile([128, KT, W], BF16, tag="nc")
                    nc.scalar.activation(ncand[:, :, :], t2[:, :, :],
                                         TANH, bias=0.0)
                    # h = ncand + z * (h - ncand)   (SBUF-only: gpsimd)
                    d_sb = gp.tile([128, KT, W], BF16, tag="d")
                    nc.gpsimd.tensor_sub(d_sb[:], h[r][:, :, :], ncand[:])
                    nc.gpsimd.tensor_mul(d_sb[:], d_sb[:], z_sb[:])
                    nc.gpsimd.tensor_add(h[r][:, :, :], ncand[:], d_sb[:])

            # ---- final pred_24 ----
            for r in range(RT):
                px = pp.tile([128, 512], F32, tag="px")
                for k in range(KT):
                    nc.tensor.matmul(
                        px[:, 0:W], (w_er[:, k, :]), (h[r][:, k, :]),
                        start=(k == 0), stop=(k == KT - 1),
                    )
                xtf = gp.tile([1, W], BF16, tag="xtf")
                nc.vector.tensor_scalar(
                    out=xtf, in0=px[0:1, 0:W],
                    scalar1=BE_CONST[0], scalar2=0.0, op0=ADD, op1=ADD)
                nc.sync.dma_start(out=preds[r][HOR - 1:HOR, :], in_=xtf)


            # ---- epilogue: out = decay + gate * (preds - decay) ----
            for r in range(RT):
                cols = slice(r * W, (r + 1) * W)
                pdec = pp.tile([HOR, W], F32, tag="pd")
                nc.tensor.matmul(pdec[:], (wdk[:]), (lvr[0:1, cols]),
                                 start=True, stop=True)
                td = gp.tile([HOR, W], F32, tag="td")
                nc.vector.tensor_tensor(out=td, in0=preds[r][:], in1=pdec[:],
                                        op=SUB)
                nc.gpsimd.tensor_mul(td[:], td[:], gate[:, cols])
                out_sb = gp.tile([HOR, W], F32, tag="osb")
                nc.vector.tensor_tensor(out=out_sb, in0=td, in1=pdec[:], op=ADD)
                nc.sync.dma_start(out=out_d[:, cols], in_=out_sb)

    nc.finalize()
    return nc


# b_e is a host-computed constant baked into the program as an immediate; the
# module-level cell lets build_nc read it before kernel() computes it.
BE_CONST = [0.0]

_NC_CACHE = None


def _get_nc():
    global _NC_CACHE
    if _NC_CACHE is None:
        _NC_CACHE = build_nc()
    return _NC_CACHE


def kernel(features, last_value, W_ih, W_hh, b_ih, b_hh, Wl, bl, Wh, bh,
           Wg1, bg1, Wg2, bg2, log_decay):
    import ml_dtypes
    NBF = ml_dtypes.bfloat16
    features = np.asarray(features, np.float32)
    last_value = np.asarray(last_value, np.float32)
    f64 = lambda a: np.asarray(a, np.float64)
    W_ih, W_hh, b_ih, b_hh = map(f64, (W_ih, W_hh, b_ih, b_hh))
    Wl, bl, Wh, bh = map(f64, (Wl, bl, Wh, bh))
    Wg1, bg1, Wg2, bg2 = map(f64, (Wg1, bg1, Wg2, bg2))

    w_e = (Wh @ Wl)[0]                      # [256]
    b_e = float((Wh @ bl + bh)[0])
    BE_CONST[0] = b_e
    W_rz_eff = W_hh[0:512] + W_ih[0:512] @ w_e[None, :]
    b_rz_eff = b_hh[0:512] + b_ih[0:512] + W_ih[0:512, 0] * b_e
    b_rz1 = b_hh[0:512] + b_ih[0:512]
    w_in = W_ih[512:768, 0]
    b_in = b_ih[512:768]
    t = np.arange(1, HOR + 1, dtype=np.float64)
    decay_curve = np.exp(-np.exp(float(log_decay)) * t)

    def pack_kpn(arr_t):  # [D, M] -> [128, KT*M] laid out (p, k*M+n)
        kt = arr_t.reshape(KT, 128, -1)
        return np.transpose(kt, (1, 0, 2)).reshape(128, -1)

    pk = np.zeros((128, PK_TOT), NBF)
    bp = np.zeros((128, BP_TOT), np.float32)
    O = PK_OFF
    OB = BP_OFF

    def put(name, block):
        block = np.asarray(block, NBF)
        pk[:block.shape[0], O[name]:O[name] + block.shape[1]] = block

    def putb(name, block):
        block = np.asarray(block, np.float32)
        bp[:block.shape[0], OB[name]:OB[name] + block.shape[1]] = block

    put("w_rz", pack_kpn(W_rz_eff.T))
    put("w_rz1", pack_kpn(W_hh[0:512].T))
    put("w_hn", pack_kpn(W_hh[512:768].T))
    put("w_er", pack_kpn(np.repeat(w_e[:, None], 128, axis=1)))
    put("wg1", pack_kpn(Wg1.T))
    put("wirz", W_ih[0:512, 0][None, :])
    put("brb", b_rz_eff[0:256][None, :])
    put("wib1", np.stack([W_ih[0:256, 0], b_rz1[0:256]]))
    put("one", np.ones((1, W)))
    put("wg2", Wg2.T)

    putb("br", b_rz_eff[0:256].reshape(2, 128).T)
    putb("br1", b_rz1[0:256].reshape(2, 128).T)
    putb("bz", b_rz_eff[256:512].reshape(2, 128).T)
    putb("bz1", b_rz1[256:512].reshape(2, 128).T)
    putb("bhn", b_hh[512:768].reshape(KT, 128).T)
    putb("win", w_in.reshape(KT, 128).T)
    putb("binn", b_in.reshape(KT, 128).T)
    putb("bg1", bg1[:, None])
    putb("bg2", bg2[:, None])

    dlv_row = np.zeros((1, HOR + ROWS), np.float32)
    dlv_row[0, 0:HOR] = decay_curve

    feat_flat = features.reshape(B * N, D)
    lv_flat = last_value.reshape(B * N)
    in_maps = []
    for i in range(NCORES):
        rows = slice(i * ROWS, (i + 1) * ROWS)
        dlv_i = dlv_row.copy()
        dlv_i[0, HOR:] = lv_flat[rows]
        m = {
            "hT": np.ascontiguousarray(
                feat_flat[rows].T.reshape(KT, 128, ROWS)).astype(NBF),
            "wpk": pk,
            "bpk": bp,
            "lvrep": np.repeat(lv_flat[rows][None, :], 128, axis=0).astype(NBF),
            "lv1": np.stack([lv_flat[rows],
                             np.ones(ROWS, np.float32)]).astype(NBF),
            "dlv": dlv_i,
        }
        in_maps.append(m)

    nc = _get_nc()
    try:
        res = run_bass_kernel_spmd(nc, in_maps, core_ids=list(range(NCORES)))
    except Exception:
        res = run_bass_kernel_spmd(nc, in_maps, core_ids=list(range(NCORES)))
    global LAST_RESULT
    LAST_RESULT = res
    out = np.concatenate([r["out"].T for r in res.results], axis=0)
    return np.ascontiguousarray(out.reshape(B, N, HOR), np.float32)


LAST_RESULT = None


# revision 34
# speedup vs baseline: 1.0639x; 1.0011x over previous
"""Trainium2 Bass kernel for nn_AutoregressivePPRM.

Model (per sequence row): 24-step autoregressive GRU (input_size=1, hidden=256)
whose scalar input at each step is the previous step's prediction
pred = (h @ Wl.T + bl) @ Wh.T + bh  -- affine in h. We exploit that to fold the
input path into the recurrent weights:

  x_{t+1} = w_e @ h_t + b_e            (w_e = Wh@Wl [1,256], b_e scalar)
  gi_rz   = W_ih_rz * x + b_ih_rz  =>  folded: W_rz_eff = W_hh_rz + W_ih_rz @ w_e
  inn     = w_in * x_t + b_in          computed from a PE-replicated x row:
            px~ = (w_e (x) 1_128) @ h  gives x on all 128 partitions, so
            inn is a cheap per-partition tensor_scalar, not a matmul.

Layout: hidden dim on partitions, B*N rows on the free axis, bf16 storage for
h / gates / weights (fp32 PSUM accumulation), which unlocks the DVE 2x mode
for the elementwise GRU state update. z-gate sigmoid is pair-fused across two
PSUM banks with its bias pre-written into PSUM by a Pool copy; r-gate sigmoids
run in-place on PSUM with per-partition bias.
Data parallel: 16000 rows sharded 2000/core across 8 cores; weights replicated.
"""

import os

import numpy as np

import concourse.bass as bass
import concourse.tile as tile
from concourse import bacc
from concourse import mybir
from concourse.bass_utils import run_bass_kernel_spmd

B, N, D, HOR, BOT = 32, 500, 256, 24, 8
NCORES = 8
ROWS = B * N // NCORES          # 2000 rows per core
RT = 4                          # row tiles per core
W = ROWS // RT                  # 500 columns per row tile
KT = D // 128                   # 2 contraction tiles

F32 = mybir.dt.float32
F32R = mybir.dt.float32r
BF16 = mybir.dt.bfloat16

# bf16 matmul weights, packed into one [128, PK_TOT] tensor
PK_OFF = {}
_c = 0
for _n, _w in [("wg1", KT * BOT), ("w_rz1", KT * 512), ("w_hn", KT * 256),
               ("wirz", 512), ("wib1", 256), ("brb", 256), ("one", W),
               ("w_er", KT * 128), ("wg2", HOR), ("w_rz", KT * 512)]:
    PK_OFF[_n] = _c
    _c += _w
PK_TOT = _c
PK_CUT = PK_OFF["w_er"]   # first DMA chunk: prologue + step-1 weights

# f32 per-partition scalars (biases), packed into [128, BP_TOT]
BP_OFF = {}
_c = 0
for _n, _w in [("br", 2), ("br1", 2), ("bz", 2), ("bz1", 2), ("bhn", KT),
               ("win", KT), ("binn", KT), ("bg1", 1), ("bg2", 1)]:
    BP_OFF[_n] = _c
    _c += _w
BP_TOT = _c


def build_nc():
    nc = bacc.Bacc()

    dram = {}

    def param(name, shape, out=False, dt=None):
        dram[name] = nc.declare_dram_parameter(
            name, list(shape), dt or F32, isOutput=out
        )
        return dram[name]

    hT_d = param("hT", (KT, 128, ROWS), dt=BF16)
    wpk_d = param("wpk", (128, PK_TOT), dt=BF16)
    bpk_d = param("bpk", (128, BP_TOT))
    lvrep_d = param("lvrep", (128, ROWS), dt=BF16)
    lv1_d = param("lv1", (2, ROWS), dt=BF16)
    dlv_d = param("dlv", (1, HOR + ROWS), dt=F32R)
    out_d = param("out", (HOR, ROWS), out=True)

    with tile.TileContext(nc, trace_sim=bool(os.environ.get('KTRACE')),
                          pool_alloc_mode='stack') as tc:
        with (
            tc.tile_pool(name="wp", bufs=1) as wp,
            tc.tile_pool(name="hp", bufs=1) as hp,
            tc.tile_pool(name="gp", bufs=5) as gp,
            tc.tile_pool(name="pp", bufs=1, space="PSUM") as pp,
        ):
            # ---- load weights/biases ----
            wpk = wp.tile([128, PK_TOT], BF16, tag="wpk")
            nc.sync.dma_start(out=wpk[:, 0:PK_CUT], in_=wpk_d[:, 0:PK_CUT])
            nc.scalar.dma_start(out=wpk[:, PK_CUT:], in_=wpk_d[:, PK_CUT:])
            O = PK_OFF
            w_rz = wpk[:, O["w_rz"]:O["w_rz"] + KT * 512].rearrange(
                "p (k n) -> p k n", k=KT)
            w_rz1 = wpk[:, O["w_rz1"]:O["w_rz1"] + KT * 512].rearrange(
                "p (k n) -> p k n", k=KT)
            w_hn = wpk[:, O["w_hn"]:O["w_hn"] + KT * 256].rearrange(
                "p (k n) -> p k n", k=KT)
            w_er = wpk[:, O["w_er"]:O["w_er"] + KT * 128].rearrange(
                "p (k n) -> p k n", k=KT)
            wg1 = wpk[:, O["wg1"]:O["wg1"] + KT * BOT].rearrange(
                "p (k n) -> p k n", k=KT)
            wirz = wpk[0:1, O["wirz"]:O["wirz"] + 512]
            brb = wpk[0:1, O["brb"]:O["brb"] + 256]
            wib1 = wpk[0:2, O["wib1"]:O["wib1"] + 256]
            one_row = wpk[0:1, O["one"]:O["one"] + W]
            wg2 = wpk[0:BOT, O["wg2"]:O["wg2"] + HOR]

            # ---- hidden state (= features, transposed), per row tile ----
            h = [hp.tile([128, KT, W], BF16, tag=f"h{r}", name=f"h{r}")
                 for r in range(RT)]

            def load_h(r):
                for k in range(KT):
                    nc.gpsimd.dma_start(
                        out=h[r][:, k, :], in_=hT_d[k, :, r * W:(r + 1) * W]
                    )

            load_h(0)
            bpk = wp.tile([128, BP_TOT], F32, tag="bpk")
            nc.gpsimd.dma_start(out=bpk, in_=bpk_d[:])
            lvrep = wp.tile([128, ROWS], BF16, tag="lvrep")
            nc.gpsimd.dma_start(out=lvrep, in_=lvrep_d[:])
            lv1 = wp.tile([2, ROWS], BF16, tag="lv1")
            nc.gpsimd.dma_start(out=lv1, in_=lv1_d[:])
            for r in range(1, RT):
                load_h(r)
            dlv = wp.tile([1, HOR + ROWS], F32R, tag="dlv")
            nc.gpsimd.dma_start(out=dlv, in_=dlv_d[:])
            OB = BP_OFF
            br = bpk[:, OB["br"]:OB["br"] + 2]
            br1 = bpk[:, OB["br1"]:OB["br1"] + 2]
            bz = bpk[:, OB["bz"]:OB["bz"] + 2]
            bz1 = bpk[:, OB["bz1"]:OB["bz1"] + 2]
            bhn = bpk[:, OB["bhn"]:OB["bhn"] + KT]
            win = bpk[:, OB["win"]:OB["win"] + KT]
            binn = bpk[:, OB["binn"]:OB["binn"] + KT]
            bg1 = bpk[0:BOT, OB["bg1"]:OB["bg1"] + 1]
            bg2 = bpk[0:HOR, OB["bg2"]:OB["bg2"] + 1]


            wdk = dlv[0:1, 0:HOR]
            lvr = dlv[0:1, HOR:HOR + ROWS]


            preds = [hp.tile([HOR - 1, W], BF16, tag=f"preds{r}",
                             name=f"preds{r}") for r in range(RT)]
            gate = hp.tile([HOR, ROWS], F32, tag="gate")

            SIG = mybir.ActivationFunctionType.Sigmoid
            TANH = mybir.ActivationFunctionType.Tanh
            ADD = mybir.AluOpType.add
            SUB = mybir.AluOpType.subtract
            MULT = mybir.AluOpType.mult
            MAXX = mybir.AluOpType.max

            # ---- prologue: mixing gate from h0 (also warms up the PE) ----
            g1 = hp.tile([BOT, ROWS], BF16, tag="g1")
            warm = ep.tile([1, 1], F32, tag="warm")
            nc.scalar.activation(warm, warm, SIG, bias=bg2[0:1, 0:1])
            bmix = hp.tile([HOR, ROWS], F32, tag="bmix")
            g23 = hp.tile([1, ROWS], F32, tag="g23")
            b23 = hp.tile([1, ROWS], F32, tag="b23")
            ncF = [hp.tile([128, KT, W], BF16, tag=f"ncF{r}", name=f"ncF{r}")
                   for r in range(RT)]
            zdF = [hp.tile([128, KT, W], BF16, tag=f"zdF{r}", name=f"zdF{r}")
                   for r in range(RT)]
            for r in range(RT):
                cols = slice(r * W, (r + 1) * W)
                pg1 = pp.tile([BOT, W], F32, tag="hn0")
                for k in range(KT):
                    nc.tensor.matmul(
                        pg1[:], (wg1[:, k, :]), (h[r][:, k, :]),
                        start=(k == 0), stop=(k == KT - 1),
                    )
                # relu(pg1 + bg1) on DVE (gpsimd cannot access PSUM)
                nc.vector.tensor_scalar(out=g1[:, cols], in0=pg1[:],
                                        scalar1=bg1[:, 0:1], scalar2=0.0,
                                        op0=ADD, op1=MAXX)
                pg2 = pp.tile([HOR, W], F32, tag="px")
                nc.tensor.matmul(pg2[:], (wg2[:]), (g1[:, cols]),
                                 start=True, stop=True)
                nc.scalar.activation(gate[:, cols], pg2[:], SIG, bias=bg2[:, 0:1])
                pdec = pp.tile([HOR, W], F32, tag="pd")
                nc.tensor.matmul(pdec[:], (wdk[:]), (lvr[0:1, cols]),
                                 start=True, stop=True)
                # B = (1 - gate) * decay, finale then needs only preds
                gm = gp.tile([HOR, W], F32, tag="gm")
                nc.vector.tensor_scalar(out=gm, in0=gate[:, cols],
                                        scalar1=-1.0, scalar2=-1.0,
                                        op0=MULT, op1=SUB)
                nc.vector.tensor_tensor(out=bmix[:, cols], in0=gm,
                                        in1=pdec[:], op=MULT)

            # ---- 24 GRU steps ----
            for u in range(1, HOR + 1):
                first = u == 1
                w_cur = w_rz1 if first else w_rz
                br_cur = br1 if first else br
                bz_cur = bz1 if first else bz
                zsel = 0 if first else 1
                for r in range(RT):
                    cols = slice(r * W, (r + 1) * W)
                    # r/z gates: one 1-bank psum tile per 128-row block so
                    # each frees as soon as its consumer has read it
                    prz = [pp.tile([128, 512], F32, tag=f"rz{m}", name=f"prz{m}")
                           for m in range(4)]
                    for m in range(4):
                        dst = prz[m][:, 0:W]
                        for k in range(KT):
                            nc.tensor.matmul(
                                dst,
                                (w_cur[:, k, m * 128:(m + 1) * 128]),
                                (h[r][:, k, :]),
                                start=(k == 0),
                                stop=(k == KT - 1 and not first),
                            )
                        if first:
                            nc.tensor.matmul(
                                dst,
                                (wirz[0:1, m * 128:(m + 1) * 128]),
                                (lvrep[0:1, cols]), start=False, stop=True,
                            )
                    # r-gate: per-bank sigmoid -> bf16 SBUF (walrus forbids
                    # two PSUM inputs on one op, so r can't stay in PSUM)
                    r_sb = gp.tile([128, KT, W], BF16, tag="rs")
                    for m in range(2):
                        nc.scalar.activation(r_sb[:, m, :], prz[m][:, 0:W],
                                             SIG, bias=br_cur[:, m:m + 1])
                    # z-gate: per-bank sigmoid -> bf16 SBUF
                    z_sb = gp.tile([128, KT, W], BF16, tag="zs")
                    for m in range(2):
                        nc.scalar.activation(z_sb[:, m, :], prz[2 + m][:, 0:W],
                                             SIG, bias=bz_cur[:, m:m + 1])

                    # hn matmuls + replicated-x matmul
                    phn = [pp.tile([128, 512], F32, tag=f"hn{m}", name=f"phn{m}")
                           for m in range(KT)]
                    for m in range(KT):
                        for k in range(KT):
                            nc.tensor.matmul(
                                phn[m][:, 0:W],
                                (w_hn[:, k, m * 128:(m + 1) * 128]),
                                (h[r][:, k, :]),
                                start=(k == 0), stop=(k == KT - 1),
                            )
                    if not first:
                        px = pp.tile([128, 512], F32, tag="px")
                        for k in range(KT):
                            nc.tensor.matmul(
                                px[:, 0:W], (w_er[:, k, :]), (h[r][:, k, :]),
                                start=(k == 0), stop=(k == KT - 1),
                            )
                        # x~ = px + b_e -> bf16 SBUF (x replicated on 128 parts)
                        xt = gp.tile([128, W], BF16, tag="xt")
                        nc.vector.tensor_scalar(
                            out=xt, in0=px[:, 0:W],
                            scalar1=BE_CONST[0], scalar2=0.0, op0=ADD, op1=ADD)
                        # pred_{u-1} = x_u  (row 0 of the replicated x~);
                        # engines can't write at a partition offset -> DMA
                        nc.sync.dma_start(out=preds[r][u - 2:u - 1, :],
                                          in_=xt[0:1, :])
                    else:
                        xt = lvrep[:, cols]

                    # t_k = (phn_k + bhn_k) * r_k   (r lives in PSUM)
                    t_sb = gp.tile([128, KT, W], BF16, tag="t")
                    for k in range(KT):
                        nc.vector.scalar_tensor_tensor(
                            out=t_sb[:, k, :], in0=phn[k][:, 0:W],
                            scalar=bhn[:, k:k + 1], in1=r_sb[:, k, :],
                            op0=ADD, op1=MULT,
                        )
                    # q_k = w_in_k * x~ + b_in_k   (all-SBUF bf16: DVE 4x mode)
                    q_sb = gp.tile([128, KT, W], BF16, tag="q")
                    for k in range(KT):
                        nc.vector.tensor_scalar(
                            out=q_sb[:, k, :], in0=xt,
                            scalar1=win[:, k:k + 1], scalar2=binn[:, k:k + 1],
                            op0=MULT, op1=ADD)
                    t2 = gp.tile([128, KT, W], BF16, tag="t2")
                    nc.vector.tensor_tensor(out=t2, in0=t_sb, in1=q_sb, op=ADD)
                    ncand = gp.tile([128, KT, W], BF16, tag="nc")
                    nc.scalar.activation(ncand[:, :, :], t2[:, :, :],
                                         TANH, bias=0.0)
                    # h = ncand + z * (h - ncand)   (SBUF-only: gpsimd)
                    d_sb = gp.tile([128, KT, W], BF16, tag="d")
                    nc.gpsimd.tensor_sub(d_sb[:], h[r][:, :, :], ncand[:])
                    nc.gpsimd.tensor_mul(d_sb[:], d_sb[:], z_sb[:])
                    nc.gpsimd.tensor_add(h[r][:, :, :], ncand[:], d_sb[:])

            # ---- final pred_24 ----
            for r in range(RT):
                px = pp.tile([128, 512], F32, tag="px")
                for k in range(KT):
                    nc.tensor.matmul(
                        px[:, 0:W], (w_er[:, k, :]), (h[r][:, k, :]),
                        start=(k == 0), stop=(k == KT - 1),
                    )
                xtf = gp.tile([1, W], BF16, tag="xtf")
                nc.vector.tensor_scalar(
                    out=xtf, in0=px[0:1, 0:W],
                    scalar1=BE_CONST[0], scalar2=0.0, op0=ADD, op1=ADD)
                nc.sync.dma_start(out=preds[r][HOR - 1:HOR, :], in_=xtf)


            # ---- epilogue: out = decay + gate * (preds - decay) ----
            for r in range(RT):
                cols = slice(r * W, (r + 1) * W)
                pdec = pp.tile([HOR, W], F32, tag="pd")
                nc.tensor.matmul(pdec[:], (wdk[:]), (lvr[0:1, cols]),
                                 start=True, stop=True)
                td = gp.tile([HOR, W], F32, tag="td")
                nc.vector.tensor_tensor(out=td, in0=preds[r][:], in1=pdec[:],
                                        op=SUB)
                nc.gpsimd.tensor_mul(td[:], td[:], gate[:, cols])
                out_sb = gp.tile([HOR, W], F32, tag="osb")
                nc.vector.tensor_tensor(out=out_sb, in0=td, in1=pdec[:], op=ADD)
                nc.sync.dma_start(out=out_d[:, cols], in_=out_sb)

    nc.finalize()
    return nc


# b_e is a host-computed constant baked into the program as an immediate; the
# module-level cell lets build_nc read it before kernel() computes it.
BE_CONST = [0.0]

_NC_CACHE = None


def _get_nc():
    global _NC_CACHE
    if _NC_CACHE is None:
        _NC_CACHE = build_nc()
    return _NC_CACHE


def kernel(features, last_value, W_ih, W_hh, b_ih, b_hh, Wl, bl, Wh, bh,
           Wg1, bg1, Wg2, bg2, log_decay):
    import ml_dtypes
    NBF = ml_dtypes.bfloat16
    features = np.asarray(features, np.float32)
    last_value = np.asarray(last_value, np.float32)
    f64 = lambda a: np.asarray(a, np.float64)
    W_ih, W_hh, b_ih, b_hh = map(f64, (W_ih, W_hh, b_ih, b_hh))
    Wl, bl, Wh, bh = map(f64, (Wl, bl, Wh, bh))
    Wg1, bg1, Wg2, bg2 = map(f64, (Wg1, bg1, Wg2, bg2))

    w_e = (Wh @ Wl)[0]                      # [256]
    b_e = float((Wh @ bl + bh)[0])
    BE_CONST[0] = b_e
    W_rz_eff = W_hh[0:512] + W_ih[0:512] @ w_e[None, :]
    b_rz_eff = b_hh[0:512] + b_ih[0:512] + W_ih[0:512, 0] * b_e
    b_rz1 = b_hh[0:512] + b_ih[0:512]
    w_in = W_ih[512:768, 0]
    b_in = b_ih[512:768]
    t = np.arange(1, HOR + 1, dtype=np.float64)
    decay_curve = np.exp(-np.exp(float(log_decay)) * t)

    def pack_kpn(arr_t):  # [D, M] -> [128, KT*M] laid out (p, k*M+n)
        kt = arr_t.reshape(KT, 128, -1)
        return np.transpose(kt, (1, 0, 2)).reshape(128, -1)

    pk = np.zeros((128, PK_TOT), NBF)
    bp = np.zeros((128, BP_TOT), np.float32)
    O = PK_OFF
    OB = BP_OFF

    def put(name, block):
        block = np.asarray(block, NBF)
        pk[:block.shape[0], O[name]:O[name] + block.shape[1]] = block

    def putb(name, block):
        block = np.asarray(block, np.float32)
        bp[:block.shape[0], OB[name]:OB[name] + block.shape[1]] = block

    put("w_rz", pack_kpn(W_rz_eff.T))
    put("w_rz1", pack_kpn(W_hh[0:512].T))
    put("w_hn", pack_kpn(W_hh[512:768].T))
    put("w_er", pack_kpn(np.repeat(w_e[:, None], 128, axis=1)))
    put("wg1", pack_kpn(Wg1.T))
    put("wirz", W_ih[0:512, 0][None, :])
    put("brb", b_rz_eff[0:256][None, :])
    put("wib1", np.stack([W_ih[0:256, 0], b_rz1[0:256]]))
    put("one", np.ones((1, W)))
    put("wg2", Wg2.T)

    putb("br", b_rz_eff[0:256].reshape(2, 128).T)
    putb("br1", b_rz1[0:256].reshape(2, 128).T)
    putb("bz", b_rz_eff[256:512].reshape(2, 128).T)
    putb("bz1", b_rz1[256:512].reshape(2, 128).T)
    putb("bhn", b_hh[512:768].reshape(KT, 128).T)
    putb("win", w_in.reshape(KT, 128).T)
    putb("binn", b_in.reshape(KT, 128).T)
    putb("bg1", bg1[:, None])
    putb("bg2", bg2[:, None])

    dlv_row = np.zeros((1, HOR + ROWS), np.float32)
    dlv_row[0, 0:HOR] = decay_curve

    feat_flat = features.reshape(B * N, D)
    lv_flat = last_value.reshape(B * N)
    in_maps = []
    for i in range(NCORES):
        rows = slice(i * ROWS, (i + 1) * ROWS)
        dlv_i = dlv_row.copy()
        dlv_i[0, HOR:] = lv_flat[rows]
        m = {
            "hT": np.ascontiguousarray(
                feat_flat[rows].T.reshape(KT, 128, ROWS)).astype(NBF),
            "wpk": pk,
            "bpk": bp,
            "lvrep": np.repeat(lv_flat[rows][None, :], 128, axis=0).astype(NBF),
            "lv1": np.stack([lv_flat[rows],
                             np.ones(ROWS, np.float32)]).astype(NBF),
            "dlv": dlv_i,
        }
        in_maps.append(m)

    nc = _get_nc()
    try:
        res = run_bass_kernel_spmd(nc, in_maps, core_ids=list(range(NCORES)))
    except Exception:
        res = run_bass_kernel_spmd(nc, in_maps, core_ids=list(range(NCORES)))
    global LAST_RESULT
    LAST_RESULT = res
    out = np.concatenate([r["out"].T for r in res.results], axis=0)
    return np.ascontiguousarray(out.reshape(B, N, HOR), np.float32)


LAST_RESULT = None


# revision 37
# speedup vs baseline: 1.0649x; 1.0009x over previous
"""Trainium2 Bass kernel for nn_AutoregressivePPRM.

Model (per sequence row): 24-step autoregressive GRU (input_size=1, hidden=256)
whose scalar input at each step is the previous step's prediction
pred = (h @ Wl.T + bl) @ Wh.T + bh  -- affine in h. We exploit that to fold the
input path into the recurrent weights:

  x_{t+1} = w_e @ h_t + b_e            (w_e = Wh@Wl [1,256], b_e scalar)
  gi_rz   = W_ih_rz * x + b_ih_rz  =>  folded: W_rz_eff = W_hh_rz + W_ih_rz @ w_e
  inn     = w_in * x_t + b_in          computed from a PE-replicated x row:
            px~ = (w_e (x) 1_128) @ h  gives x on all 128 partitions, so
            inn is a cheap per-partition tensor_scalar, not a matmul.

Layout: hidden dim on partitions, B*N rows on the free axis, bf16 storage for
h / gates / weights (fp32 PSUM accumulation), which unlocks the DVE 2x mode
for the elementwise GRU state update. z-gate sigmoid is pair-fused across two
PSUM banks with its bias pre-written into PSUM by a Pool copy; r-gate sigmoids
run in-place on PSUM with per-partition bias.
Data parallel: 16000 rows sharded 2000/core across 8 cores; weights replicated.
"""

import os

import numpy as np

import concourse.bass as bass
import concourse.tile as tile
from concourse import bacc
from concourse import mybir
from concourse.bass_utils import run_bass_kernel_spmd

B, N, D, HOR, BOT = 32, 500, 256, 24, 8
NCORES = 8
ROWS = B * N // NCORES          # 2000 rows per core
RT = 4                          # row tiles per core
W = ROWS // RT                  # 500 columns per row tile
KT = D // 128                   # 2 contraction tiles

F32 = mybir.dt.float32
F32R = mybir.dt.float32r
BF16 = mybir.dt.bfloat16

# bf16 matmul weights, packed into one [128, PK_TOT] tensor
PK_OFF = {}
_c = 0
for _n, _w in [("wg1", KT * BOT), ("w_rz1", KT * 512), ("w_hn", KT * 256),
               ("wirz", 512), ("wib1", 256), ("brb", 256), ("one", W),
               ("w_er", KT * 128), ("wg2", HOR), ("w_rz", KT * 512)]:
    PK_OFF[_n] = _c
    _c += _w
PK_TOT = _c
PK_CUT = PK_OFF["w_er"]   # first DMA chunk: prologue + step-1 weights

# f32 per-partition scalars (biases), packed into [128, BP_TOT]
BP_OFF = {}
_c = 0
for _n, _w in [("br", 2), ("br1", 2), ("bz", 2), ("bz1", 2), ("bhn", KT),
               ("win", KT), ("binn", KT), ("bg1", 1), ("bg2", 1)]:
    BP_OFF[_n] = _c
    _c += _w
BP_TOT = _c


def build_nc():
    nc = bacc.Bacc()

    dram = {}

    def param(name, shape, out=False, dt=None):
        dram[name] = nc.declare_dram_parameter(
            name, list(shape), dt or F32, isOutput=out
        )
        return dram[name]

    hT_d = param("hT", (KT, 128, ROWS), dt=BF16)
    wpk_d = param("wpk", (128, PK_TOT), dt=BF16)
    bpk_d = param("bpk", (128, BP_TOT))
    lvrep_d = param("lvrep", (128, ROWS), dt=BF16)
    lv1_d = param("lv1", (2, ROWS), dt=BF16)
    dlv_d = param("dlv", (1, HOR + ROWS), dt=F32R)
    out_d = param("out", (HOR, ROWS), out=True)

    with tile.TileContext(nc, trace_sim=bool(os.environ.get('KTRACE')),
                          pool_alloc_mode='stack') as tc:
        with (
            tc.tile_pool(name="wp", bufs=1) as wp,
            tc.tile_pool(name="hp", bufs=1) as hp,
            tc.tile_pool(name="gp", bufs=5) as gp,
            tc.tile_pool(name="pp", bufs=1, space="PSUM") as pp,
        ):
            # ---- load weights/biases ----
            wpk = wp.tile([128, PK_TOT], BF16, tag="wpk")
            nc.sync.dma_start(out=wpk[:, 0:PK_CUT], in_=wpk_d[:, 0:PK_CUT])
            nc.scalar.dma_start(out=wpk[:, PK_CUT:], in_=wpk_d[:, PK_CUT:])
            O = PK_OFF
            w_rz = wpk[:, O["w_rz"]:O["w_rz"] + KT * 512].rearrange(
                "p (k n) -> p k n", k=KT)
            w_rz1 = wpk[:, O["w_rz1"]:O["w_rz1"] + KT * 512].rearrange(
                "p (k n) -> p k n", k=KT)
            w_hn = wpk[:, O["w_hn"]:O["w_hn"] + KT * 256].rearrange(
                "p (k n) -> p k n", k=KT)
            w_er = wpk[:, O["w_er"]:O["w_er"] + KT * 128].rearrange(
                "p (k n) -> p k n", k=KT)
            wg1 = wpk[:, O["wg1"]:O["wg1"] + KT * BOT].rearrange(
                "p (k n) -> p k n", k=KT)
            wirz = wpk[0:1, O["wirz"]:O["wirz"] + 512]
            brb = wpk[0:1, O["brb"]:O["brb"] + 256]
            wib1 = wpk[0:2, O["wib1"]:O["wib1"] + 256]
            one_row = wpk[0:1, O["one"]:O["one"] + W]
            wg2 = wpk[0:BOT, O["wg2"]:O["wg2"] + HOR]

            # ---- hidden state (= features, transposed), per row tile ----
            h = [hp.tile([128, KT, W], BF16, tag=f"h{r}", name=f"h{r}")
                 for r in range(RT)]

            def load_h(r):
                for k in range(KT):
                    nc.gpsimd.dma_start(
                        out=h[r][:, k, :], in_=hT_d[k, :, r * W:(r + 1) * W]
                    )

            load_h(0)
            bpk = wp.tile([128, BP_TOT], F32, tag="bpk")
            nc.gpsimd.dma_start(out=bpk, in_=bpk_d[:])
            lvrep = wp.tile([128, ROWS], BF16, tag="lvrep")
            nc.gpsimd.dma_start(out=lvrep, in_=lvrep_d[:])
            lv1 = wp.tile([2, ROWS], BF16, tag="lv1")
            nc.gpsimd.dma_start(out=lv1, in_=lv1_d[:])
            for r in range(1, RT):
                load_h(r)
            dlv = wp.tile([1, HOR + ROWS], F32R, tag="dlv")
            nc.gpsimd.dma_start(out=dlv, in_=dlv_d[:])
            OB = BP_OFF
            br = bpk[:, OB["br"]:OB["br"] + 2]
            br1 = bpk[:, OB["br1"]:OB["br1"] + 2]
            bz = bpk[:, OB["bz"]:OB["bz"] + 2]
            bz1 = bpk[:, OB["bz1"]:OB["bz1"] + 2]
            bhn = bpk[:, OB["bhn"]:OB["bhn"] + KT]
            win = bpk[:, OB["win"]:OB["win"] + KT]
            binn = bpk[:, OB["binn"]:OB["binn"] + KT]
            bg1 = bpk[0:BOT, OB["bg1"]:OB["bg1"] + 1]
            bg2 = bpk[0:HOR, OB["bg2"]:OB["bg2"] + 1]


            wdk = dlv[0:1, 0:HOR]
            lvr = dlv[0:1, HOR:HOR + ROWS]


            preds = [hp.tile([HOR - 1, W], BF16, tag=f"preds{r}",
                             name=f"preds{r}") for r in range(RT)]
            gate = hp.tile([HOR, ROWS], F32, tag="gate")

            SIG = mybir.ActivationFunctionType.Sigmoid
            TANH = mybir.ActivationFunctionType.Tanh
            ADD = mybir.AluOpType.add
            SUB = mybir.AluOpType.subtract
            MULT = mybir.AluOpType.mult
            MAXX = mybir.AluOpType.max

            # ---- prologue: mixing gate from h0 (also warms up the PE) ----
            g1 = hp.tile([BOT, ROWS], BF16, tag="g1")
            warm = ep.tile([1, 1], F32, tag="warm")
            nc.scalar.activation(warm, warm, SIG, bias=bg2[0:1, 0:1])
            bmix = hp.tile([HOR, ROWS], F32, tag="bmix")
            g23 = hp.tile([1, ROWS], F32, tag="g23")
            b23 = hp.tile([1, ROWS], F32, tag="b23")
            ncF = [hp.tile([128, KT, W], BF16, tag=f"ncF{r}", name=f"ncF{r}")
                   for r in range(RT)]
            zdF = [hp.tile([128, KT, W], BF16, tag=f"zdF{r}", name=f"zdF{r}")
                   for r in range(RT)]
            for r in range(RT):
                cols = slice(r * W, (r + 1) * W)
                pg1 = pp.tile([BOT, W], F32, tag="hn0")
                for k in range(KT):
                    nc.tensor.matmul(
                        pg1[:], (wg1[:, k, :]), (h[r][:, k, :]),
                        start=(k == 0), stop=(k == KT - 1),
                    )
                # relu(pg1 + bg1) on DVE (gpsimd cannot access PSUM)
                nc.vector.tensor_scalar(out=g1[:, cols], in0=pg1[:],
                                        scalar1=bg1[:, 0:1], scalar2=0.0,
                                        op0=ADD, op1=MAXX)
                pg2 = pp.tile([HOR, W], F32, tag="px0")
                nc.tensor.matmul(pg2[:], (wg2[:]), (g1[:, cols]),
                                 start=True, stop=True)
                nc.scalar.activation(gate[:, cols], pg2[:], SIG, bias=bg2[:, 0:1])
                pdec = pp.tile([HOR, W], F32, tag="px1")
                nc.tensor.matmul(pdec[:], (wdk[:]), (lvr[0:1, cols]),
                                 start=True, stop=True)
                # B = (1 - gate) * decay, finale then needs only preds
                gm = gp.tile([HOR, W], F32, tag="gm")
                nc.vector.tensor_scalar(out=gm, in0=gate[:, cols],
                                        scalar1=-1.0, scalar2=-1.0,
                                        op0=MULT, op1=SUB)
                nc.vector.tensor_tensor(out=bmix[:, cols], in0=gm,
                                        in1=pdec[:], op=MULT)

            # ---- 24 GRU steps ----
            for u in range(1, HOR + 1):
                first = u == 1
                w_cur = w_rz1 if first else w_rz
                br_cur = br1 if first else br
                bz_cur = bz1 if first else bz
                zsel = 0 if first else 1
                for r in range(RT):
                    cols = slice(r * W, (r + 1) * W)
                    # r/z gates: one 1-bank psum tile per 128-row block so
                    # each frees as soon as its consumer has read it
                    prz = [pp.tile([128, 512], F32, tag=f"rz{m}", name=f"prz{m}")
                           for m in range(4)]
                    for m in range(4):
                        dst = prz[m][:, 0:W]
                        for k in range(KT):
                            nc.tensor.matmul(
                                dst,
                                (w_cur[:, k, m * 128:(m + 1) * 128]),
                                (h[r][:, k, :]),
                                start=(k == 0),
                                stop=(k == KT - 1 and not first),
                            )
                        if first:
                            nc.tensor.matmul(
                                dst,
                                (wirz[0:1, m * 128:(m + 1) * 128]),
                                (lvrep[0:1, cols]), start=False, stop=True,
                            )
                    # r-gate: per-bank sigmoid -> bf16 SBUF (walrus forbids
                    # two PSUM inputs on one op, so r can't stay in PSUM)
                    r_sb = gp.tile([128, KT, W], BF16, tag="rs")
                    for m in range(2):
                        nc.scalar.activation(r_sb[:, m, :], prz[m][:, 0:W],
                                             SIG, bias=br_cur[:, m:m + 1])
                    # z-gate: per-bank sigmoid -> bf16 SBUF
                    z_sb = gp.tile([128, KT, W], BF16, tag="zs")
                    for m in range(2):
                        nc.scalar.activation(z_sb[:, m, :], prz[2 + m][:, 0:W],
                                             SIG, bias=bz_cur[:, m:m + 1])

                    # hn matmuls + replicated-x matmul
                    phn = [pp.tile([128, 512], F32, tag=f"hn{m}", name=f"phn{m}")
                           for m in range(KT)]
                    for m in range(KT):
                        for k in range(KT):
                            nc.tensor.matmul(
                                phn[m][:, 0:W],
                                (w_hn[:, k, m * 128:(m + 1) * 128]),
                                (h[r][:, k, :]),
                                start=(k == 0), stop=(k == KT - 1),
                            )
                    if not first:
                        px = pp.tile([128, 512], F32, tag="px")
                        for k in range(KT):
                            nc.tensor.matmul(
                                px[:, 0:W], (w_er[:, k, :]), (h[r][:, k, :]),
                                start=(k == 0), stop=(k == KT - 1),
                            )
                        # x~ = px + b_e -> bf16 SBUF (x replicated on 128 parts)
                        xt = gp.tile([128, W], BF16, tag="xt")
                        nc.vector.tensor_scalar(
                            out=xt, in0=px[:, 0:W],
                            scalar1=BE_CONST[0], scalar2=0.0, op0=ADD, op1=ADD)
                        # pred_{u-1} = x_u  (row 0 of the replicated x~);
                        # engines can't write at a partition offset -> DMA
                        nc.sync.dma_start(out=preds[r][u - 2:u - 1, :],
                                          in_=xt[0:1, :])
                    else:
                        xt = lvrep[:, cols]

                    # t_k = (phn_k + bhn_k) * r_k   (r lives in PSUM)
                    t_sb = gp.tile([128, KT, W], BF16, tag="t")
                    for k in range(KT):
                        nc.vector.scalar_tensor_tensor(
                            out=t_sb[:, k, :], in0=phn[k][:, 0:W],
                            scalar=bhn[:, k:k + 1], in1=r_sb[:, k, :],
                            op0=ADD, op1=MULT,
                        )
                    # q_k = w_in_k * x~ + b_in_k   (all-SBUF bf16: DVE 4x mode)
                    q_sb = gp.tile([128, KT, W], BF16, tag="q")
                    for k in range(KT):
                        nc.vector.tensor_scalar(
                            out=q_sb[:, k, :], in0=xt,
                            scalar1=win[:, k:k + 1], scalar2=binn[:, k:k + 1],
                            op0=MULT, op1=ADD)
                    t2 = gp.tile([128, KT, W], BF16, tag="t2")
                    nc.vector.tensor_tensor(out=t2, in0=t_sb, in1=q_sb, op=ADD)
                    ncand = gp.tile([128, KT, W], BF16, tag="nc")
                    nc.scalar.activation(ncand[:, :, :], t2[:, :, :],
                                         TANH, bias=0.0)
                    # h = ncand + z * (h - ncand)   (SBUF-only: gpsimd)
                    d_sb = gp.tile([128, KT, W], BF16, tag="d")
                    nc.gpsimd.tensor_sub(d_sb[:], h[r][:, :, :], ncand[:])
                    nc.gpsimd.tensor_mul(d_sb[:], d_sb[:], z_sb[:])
                    nc.gpsimd.tensor_add(h[r][:, :, :], ncand[:], d_sb[:])

            # ---- final pred_24 ----
            for r in range(RT):
                px = pp.tile([128, 512], F32, tag="px")
                for k in range(KT):
                    nc.tensor.matmul(
                        px[:, 0:W], (w_er[:, k, :]), (h[r][:, k, :]),
                        start=(k == 0), stop=(k == KT - 1),
                    )
                xtf = gp.tile([1, W], BF16, tag="xtf")
                nc.vector.tensor_scalar(
                    out=xtf, in0=px[0:1, 0:W],
                    scalar1=BE_CONST[0], scalar2=0.0, op0=ADD, op1=ADD)
                nc.sync.dma_start(out=preds[r][HOR - 1:HOR, :], in_=xtf)


            # ---- epilogue: out = decay + gate * (preds - decay) ----
            for r in range(RT):
                cols = slice(r * W, (r + 1) * W)
                pdec = pp.tile([HOR, W], F32, tag="px1")
                nc.tensor.matmul(pdec[:], (wdk[:]), (lvr[0:1, cols]),
                                 start=True, stop=True)
                td = gp.tile([HOR, W], F32, tag="td")
                nc.vector.tensor_tensor(out=td, in0=preds[r][:], in1=pdec[:],
                                        op=SUB)
                nc.gpsimd.tensor_mul(td[:], td[:], gate[:, cols])
                out_sb = gp.tile([HOR, W], F32, tag="osb")
                nc.vector.tensor_tensor(out=out_sb, in0=td, in1=pdec[:], op=ADD)
                nc.sync.dma_start(out=out_d[:, cols], in_=out_sb)

    nc.finalize()
    return nc


# b_e is a host-computed constant baked into the program as an immediate; the
# module-level cell lets build_nc read it before kernel() computes it.
BE_CONST = [0.0]

_NC_CACHE = None


def _get_nc():
    global _NC_CACHE
    if _NC_CACHE is None:
        _NC_CACHE = build_nc()
    return _NC_CACHE


def kernel(features, last_value, W_ih, W_hh, b_ih, b_hh, Wl, bl, Wh, bh,
           Wg1, bg1, Wg2, bg2, log_decay):
    import ml_dtypes
    NBF = ml_dtypes.bfloat16
    features = np.asarray(features, np.float32)
    last_value = np.asarray(last_value, np.float32)
    f64 = lambda a: np.asarray(a, np.float64)
    W_ih, W_hh, b_ih, b_hh = map(f64, (W_ih, W_hh, b_ih, b_hh))
    Wl, bl, Wh, bh = map(f64, (Wl, bl, Wh, bh))
    Wg1, bg1, Wg2, bg2 = map(f64, (Wg1, bg1, Wg2, bg2))

    w_e = (Wh @ Wl)[0]                      # [256]
    b_e = float((Wh @ bl + bh)[0])
    BE_CONST[0] = b_e
    W_rz_eff = W_hh[0:512] + W_ih[0:512] @ w_e[None, :]
    b_rz_eff = b_hh[0:512] + b_ih[0:512] + W_ih[0:512, 0] * b_e
    b_rz1 = b_hh[0:512] + b_ih[0:512]
    w_in = W_ih[512:768, 0]
    b_in = b_ih[512:768]
    t = np.arange(1, HOR + 1, dtype=np.float64)
    decay_curve = np.exp(-np.exp(float(log_decay)) * t)

    def pack_kpn(arr_t):  # [D, M] -> [128, KT*M] laid out (p, k*M+n)
        kt = arr_t.reshape(KT, 128, -1)
        return np.transpose(kt, (1, 0, 2)).reshape(128, -1)

    pk = np.zeros((128, PK_TOT), NBF)
    bp = np.zeros((128, BP_TOT), np.float32)
    O = PK_OFF
    OB = BP_OFF

    def put(name, block):
        block = np.asarray(block, NBF)
        pk[:block.shape[0], O[name]:O[name] + block.shape[1]] = block

    def putb(name, block):
        block = np.asarray(block, np.float32)
        bp[:block.shape[0], OB[name]:OB[name] + block.shape[1]] = block

    put("w_rz", pack_kpn(W_rz_eff.T))
    put("w_rz1", pack_kpn(W_hh[0:512].T))
    put("w_hn", pack_kpn(W_hh[512:768].T))
    put("w_er", pack_kpn(np.repeat(w_e[:, None], 128, axis=1)))
    put("wg1", pack_kpn(Wg1.T))
    put("wirz", W_ih[0:512, 0][None, :])
    put("brb", b_rz_eff[0:256][None, :])
    put("wib1", np.stack([W_ih[0:256, 0], b_rz1[0:256]]))
    put("one", np.ones((1, W)))
    put("wg2", Wg2.T)

    putb("br", b_rz_eff[0:256].reshape(2, 128).T)
    putb("br1", b_rz1[0:256].reshape(2, 128).T)
    putb("bz", b_rz_eff[256:512].reshape(2, 128).T)
    putb("bz1", b_rz1[256:512].reshape(2, 128).T)
    putb("bhn", b_hh[512:768].reshape(KT, 128).T)
    putb("win", w_in.reshape(KT, 128).T)
    putb("binn", b_in.reshape(KT, 128).T)
    putb("bg1", bg1[:, None])
    putb("bg2", bg2[:, None])

    dlv_row = np.zeros((1, HOR + ROWS), np.float32)
    dlv_row[0, 0:HOR] = decay_curve

    feat_flat = features.reshape(B * N, D)
    lv_flat = last_value.reshape(B * N)
    in_maps = []
    for i in range(NCORES):
        rows = slice(i * ROWS, (i + 1) * ROWS)
        dlv_i = dlv_row.copy()
        dlv_i[0, HOR:] = lv_flat[rows]
        m = {
            "hT": np.ascontiguousarray(
                feat_flat[rows].T.reshape(KT, 128, ROWS)).astype(NBF),
            "wpk": pk,
            "bpk": bp,
            "lvrep": np.repeat(lv_flat[rows][None, :], 128, axis=0).astype(NBF),
            "lv1": np.stack([lv_flat[rows],
                             np.ones(ROWS, np.float32)]).astype(NBF),
            "dlv": dlv_i,
        }
        in_maps.append(m)

    nc = _get_nc()
    try:
        res = run_bass_kernel_spmd(nc, in_maps, core_ids=list(range(NCORES)))
    except Exception:
        res = run_bass_kernel_spmd(nc, in_maps, core_ids=list(range(NCORES)))
    global LAST_RESULT
    LAST_RESULT = res
    out = np.concatenate([r["out"].T for r in res.results], axis=0)
    return np.ascontiguousarray(out.reshape(B, N, HOR), np.float32)


LAST_RESULT = None
